# revision 1
# baseline (speedup 1.0000x reference)
"""Trainium2 Bass kernel for nn_BClassifier (MIL attention pooling).

Reference computation (per bag b of BATCH=4, INST=40000 instances, DIM=512):
    Q = tanh(relu(feats @ w1 + b1) @ w2 + b2)                 # [n, 128]
    top = argmax(c[:, o]) per class o                          # instance index
    q_max = q_mlp(feats[top])                                  # [2, 128]
    A = softmax(Q @ q_max.T / sqrt(n), axis=instances)         # [n, 2]
    B = A.T @ feats                                            # [2, 512]
    C = einsum('id,oid->o', B, fcc_w) + fcc_b                  # [2]
    returns (C, A, B)

Sharding: bags x halves -> 8 cores. Core c handles bag b=c//2, instance half
h=c%2 (20000 instances). The softmax normalizer Z and the unnormalized
B = sum_n exp(s_n) * feats_n are partial-summed per core and combined with a
2-rank AllReduce per bag pair; each core then normalizes its half of A on
device. The argmax of c (tiny index computation) is resolved at shard time on
the host, which replicates the winning instances' features to both cores of
the bag (the device runs the q_mlp on them).

Compute runs in bf16 on the TensorEngine (accumulation in fp32 PSUM). Since
the PE contracts over the partition axis, the MLP needs feats with the feature
dim on partitions while the B-accumulation needs the instance dim on
partitions; the host therefore ships the per-core feats slice in both layouts
(bf16), which costs the same HBM bytes as a single f32 layout: the kernel
stays at the ~41MB/core memory roofline.

exp(s) needs no max subtraction: |s| <= 128/sqrt(40000) = 0.64 by construction
(tanh-bounded dot products), so softmax is computed as exp(s)/sum(exp(s)).
"""

import copy

import numpy as np
import ml_dtypes

import concourse.bass as bass
import concourse.mybir as mybir
import concourse.tile as tile
from concourse.bass_utils import run_bass_kernel_spmd

BATCH = 4
INST = 40000
DIM = 512
HID = 128
OC = 2
N_CORES = 8

N_LOC = INST // 2            # 20000 instances per core
N_TILES = 157                # ceil(20000 / 128)
N_PAD = N_TILES * 128        # 20096
LAST_ROWS = N_LOC - (N_TILES - 1) * 128   # 32
SCALE = 1.0 / np.sqrt(np.float32(INST))   # 1/200

F32 = mybir.dt.float32
BF16 = mybir.dt.bfloat16

_CACHE = {}


def _split_multi_waits(nc, tmpl):
    """Walrus in this container only accepts one sync-wait per instruction.
    Split any multi-wait instruction into single-wait EventSemaphore preludes
    on the same engine (engine sequencers execute in order, so gating is
    preserved)."""
    n = 0
    for bb in nc.main_func.blocks:
        out = []
        for inst in bb.instructions:
            si = inst.sync_info
            if si is not None and si.on_wait is not None and len(si.on_wait) > 1:
                waits = list(si.on_wait)
                for w in waits[:-1]:
                    nop = copy.deepcopy(tmpl)
                    n += 1
                    nop.name = f"I-wsplit-{n}"
                    nop.engine = inst.engine
                    nop.sync_info = mybir.SyncInfo(on_wait=[w], on_update=[])
                    out.append(nop)
                inst.sync_info = mybir.SyncInfo(
                    on_wait=[waits[-1]], on_update=list(si.on_update)
                )
            out.append(inst)
        bb.instructions = out
    return n


def _build_nc():
    nc = bass.Bass("TRN2", target_bir_lowering=False, debug=False,
                   num_devices=N_CORES)

    # template wait instruction for _split_multi_waits
    with nc.semaphore() as _s:
        _ti = nc.vector.wait_ge(_s, 1)
        tmpl = copy.deepcopy(_ti.ins)
    for bb in nc.main_func.blocks:
        bb.instructions = [i for i in bb.instructions if i.name != _ti.ins.name]

    # ---- I/O ----
    hF = nc.dram_tensor("F", [N_PAD, DIM], BF16, kind="ExternalInput")
    hFT = nc.dram_tensor("FT", [DIM, N_PAD], BF16, kind="ExternalInput")
    hmT = nc.dram_tensor("mT", [DIM, OC], F32, kind="ExternalInput")
    hw1b = nc.dram_tensor("w1b", [DIM, HID], BF16, kind="ExternalInput")
    hw2b = nc.dram_tensor("w2b", [HID, HID], BF16, kind="ExternalInput")
    hw1f = nc.dram_tensor("w1f", [DIM, HID], F32, kind="ExternalInput")
    hw2f = nc.dram_tensor("w2f", [HID, HID], F32, kind="ExternalInput")
    hb1 = nc.dram_tensor("b1", [HID, 1], F32, kind="ExternalInput")
    hb2 = nc.dram_tensor("b2", [HID, 1], F32, kind="ExternalInput")
    hfw = nc.dram_tensor("fw", [OC, OC, DIM], F32, kind="ExternalInput")
    hfb = nc.dram_tensor("fb", [1, OC], F32, kind="ExternalInput")

    hA = nc.dram_tensor("A_out", [N_PAD, OC], F32, kind="ExternalOutput")
    hB = nc.dram_tensor("B_out", [OC, DIM], F32, kind="ExternalOutput")
    hC = nc.dram_tensor("C_out", [1, OC], F32, kind="ExternalOutput")

    with tile.TileContext(nc) as tc:
        with (
            tc.tile_pool(name="const", bufs=1) as cpool,
            tc.tile_pool(name="stage", bufs=1) as spool,
            tc.tile_pool(name="fb_pool", bufs=4) as fpool,
            tc.tile_pool(name="ft_pool", bufs=4) as ftpool,
            tc.tile_pool(name="act", bufs=3) as apool,
            tc.tile_pool(name="ps_h", bufs=2, space="PSUM") as ps_h,
            tc.tile_pool(name="ps_g", bufs=2, space="PSUM") as ps_g,
            tc.tile_pool(name="ps_s", bufs=2, space="PSUM") as ps_s,
            tc.tile_pool(name="ps_acc", bufs=1, space="PSUM") as ps_acc,
            tc.tile_pool(name="dram", bufs=1, space="DRAM") as dram,
        ):
            # ---- constants into SBUF ----
            w1b = cpool.tile([128, 4, HID], BF16, tag="w1b")
            nc.sync.dma_start(w1b[:], hw1b.ap().rearrange("(k p) h -> p k h", p=128))
            w2b = cpool.tile([128, HID], BF16, tag="w2b")
            nc.sync.dma_start(w2b[:], hw2b[:])
            w1f = cpool.tile([128, 4, HID], F32, tag="w1f")
            nc.sync.dma_start(w1f[:], hw1f.ap().rearrange("(k p) h -> p k h", p=128))
            w2f = cpool.tile([128, HID], F32, tag="w2f")
            nc.sync.dma_start(w2f[:], hw2f[:])
            mT = cpool.tile([128, 4, OC], F32, tag="mT")
            nc.sync.dma_start(mT[:], hmT.ap().rearrange("(k p) o -> p k o", p=128))
            b1 = cpool.tile([HID, 1], F32, tag="b1")
            nc.sync.dma_start(b1[:], hb1[:])
            b2 = cpool.tile([HID, 1], F32, tag="b2")
            nc.sync.dma_start(b2[:], hb2[:])
            fw = cpool.tile([OC, OC, DIM], F32, tag="fw")
            nc.sync.dma_start(fw[:], hfw.ap().rearrange("o i d -> i o d"))
            fbt = cpool.tile([1, OC], F32, tag="fbt")
            nc.sync.dma_start(fbt[:], hfb[:])

            ones_bf = cpool.tile([128, 1], BF16, tag="ones_bf")
            nc.vector.memset(ones_bf[:], 1.0)
            ones_r = cpool.tile([1, 128], F32, tag="ones_r")
            nc.vector.memset(ones_r[:], 1.0)
            ones_c2 = cpool.tile([2, 1], F32, tag="ones_c2")
            nc.vector.memset(ones_c2[:], 1.0)

            # A staging buffer: e = exp(s) for all local tiles, f32
            Asb = spool.tile([128, N_TILES, OC], F32, tag="Asb")

            # ---- q_max chain (f32, one-time) ----
            hm_ps = ps_h.tile([128, OC], F32, tag="h")
            for k in range(4):
                nc.tensor.matmul(hm_ps[:], lhsT=w1f[:, k, :], rhs=mT[:, k, :],
                                 start=(k == 0), stop=(k == 3))
            hm = apool.tile([128, OC], F32, tag="hm")
            nc.scalar.activation(hm[:], hm_ps[:],
                                 mybir.ActivationFunctionType.Relu, bias=b1[:])
            gm_ps = ps_g.tile([128, OC], F32, tag="g")
            nc.tensor.matmul(gm_ps[:], lhsT=w2f[:], rhs=hm[:], start=True, stop=True)
            qmT_f = apool.tile([128, OC], F32, tag="qmT_f")
            nc.scalar.activation(qmT_f[:], gm_ps[:],
                                 mybir.ActivationFunctionType.Tanh, bias=b2[:])
            qmT = cpool.tile([128, OC], BF16, tag="qmT")
            nc.vector.tensor_copy(qmT[:], qmT_f[:])

            # ---- accumulators (persist across the whole loop) ----
            B_ps = ps_acc.tile([OC, DIM], F32, tag="B")
            Z_ps = ps_acc.tile([OC, 1], F32, tag="Z")

            # ---- main loop over instance tiles ----
            FT_view = hFT.ap().rearrange("(k p) n -> p k n", p=128)
            for t in range(N_TILES):
                kk = 128 if t < N_TILES - 1 else LAST_ROWS

                Fb = fpool.tile([128, DIM], BF16, tag="Fb")
                nc.sync.dma_start(Fb[:], hF[t * 128:(t + 1) * 128, :])
                FT = ftpool.tile([128, 4, 128], BF16, tag="FT")
                nc.sync.dma_start(FT[:], FT_view[:, :, t * 128:(t + 1) * 128])

                h_ps = ps_h.tile([HID, 128], F32, tag="h")
                for k in range(4):
                    nc.tensor.matmul(h_ps[:], lhsT=w1b[:, k, :], rhs=FT[:, k, :],
                                     start=(k == 0), stop=(k == 3))
                h_bf = apool.tile([HID, 128], BF16, tag="h_bf")
                nc.scalar.activation(h_bf[:], h_ps[:],
                                     mybir.ActivationFunctionType.Relu, bias=b1[:])

                g_ps = ps_g.tile([HID, 128], F32, tag="g")
                nc.tensor.matmul(g_ps[:], lhsT=w2b[:], rhs=h_bf[:],
                                 start=True, stop=True)
                q_bf = apool.tile([HID, 128], BF16, tag="q_bf")
                nc.scalar.activation(q_bf[:], g_ps[:],
                                     mybir.ActivationFunctionType.Tanh, bias=b2[:])

                # s = Q @ q_max^T  (contraction over hidden dim on partitions)
                s_ps = ps_s.tile([128, OC], F32, tag="s")
                nc.tensor.matmul(s_ps[:], lhsT=q_bf[:], rhs=qmT[:],
                                 start=True, stop=True)

                # e = exp(s / 200): bf16 copy for the PE accumulators,
                # f32 copy staged for the A output
                e_bf = apool.tile([128, OC], BF16, tag="e_bf")
                nc.scalar.activation(e_bf[:], s_ps[:],
                                     mybir.ActivationFunctionType.Exp, scale=float(SCALE))
                nc.scalar.activation(Asb[:, t, :], s_ps[:],
                                     mybir.ActivationFunctionType.Exp, scale=float(SCALE))

                # B += e^T F ; Z += e^T 1   (contraction over instances)
                nc.tensor.matmul(B_ps[:], lhsT=e_bf[:kk, :], rhs=Fb[:kk, :],
                                 start=(t == 0), stop=(t == N_TILES - 1),
                                 skip_group_check=True)
                nc.tensor.matmul(Z_ps[:], lhsT=e_bf[:kk, :], rhs=ones_bf[:kk, :],
                                 start=(t == 0), stop=(t == N_TILES - 1),
                                 skip_group_check=True)

            # ---- combine partials across the bag's core pair ----
            B_sb = apool.tile([OC, DIM], F32, tag="B_sb")
            nc.vector.tensor_copy(B_sb[:], B_ps[:])
            Z_sb = apool.tile([OC, 1], F32, tag="Z_sb")
            nc.vector.tensor_copy(Z_sb[:], Z_ps[:])

            cc_in = dram.tile([OC, DIM + 1], F32, tag="cc_in")
            cc_out = dram.tile([OC, DIM + 1], F32, tag="cc_out")
            nc.sync.dma_start(cc_in[:, 0:1], Z_sb[:])
            nc.sync.dma_start(cc_in[:, 1:DIM + 1], B_sb[:])
            nc.gpsimd.collective_compute(
                "AllReduce",
                mybir.AluOpType.add,
                replica_groups=[[0, 1], [2, 3], [4, 5], [6, 7]],
                ins=[cc_in.opt()],
                outs=[cc_out.opt()],
            )

            Zg_c = apool.tile([OC, 1], F32, tag="Zg_c")
            nc.sync.dma_start(Zg_c[:], cc_out[:, 0:1])
            Zg_r = apool.tile([1, OC], F32, tag="Zg_r")
            nc.sync.dma_start(Zg_r[:], cc_out[:, 0:1].rearrange("o x -> x o"))
            Bg = apool.tile([OC, DIM], F32, tag="Bg")
            nc.sync.dma_start(Bg[:], cc_out[:, 1:DIM + 1])

            zi_c = apool.tile([OC, 1], F32, tag="zi_c")
            nc.vector.reciprocal(zi_c[:], Zg_c[:])
            zi_r = apool.tile([1, OC], F32, tag="zi_r")
            nc.vector.reciprocal(zi_r[:], Zg_r[:])

            # B_out = B_glob / Z  -> HBM
            Bn = apool.tile([OC, DIM], F32, tag="Bn")
            nc.vector.tensor_scalar_mul(Bn[:], Bg[:], zi_c[:])
            nc.sync.dma_start(hB[:], Bn[:])

            # A = e / Z : broadcast 1/Z down the partitions via rank-1 matmul
            zb_ps = ps_h.tile([128, OC], F32, tag="h")
            nc.tensor.matmul(zb_ps[:], lhsT=ones_r[:], rhs=zi_r[:],
                             start=True, stop=True)
            zb = apool.tile([128, OC], F32, tag="zb")
            nc.vector.tensor_copy(zb[:], zb_ps[:])
            nc.vector.tensor_mul(Asb[:], Asb[:],
                                 zb[:, None, :].broadcast_to([128, N_TILES, OC]))
            nc.sync.dma_start(hA.ap().rearrange("(t p) o -> p t o", p=128), Asb[:])

            # C = einsum('id,oid->o', B_out, fcc_w) + fcc_b
            R = apool.tile([OC, OC], F32, tag="R")
            p0 = apool.tile([OC, DIM], F32, tag="p0")
            for o in range(OC):
                nc.vector.tensor_mul(p0[:], Bn[:], fw[:, o, :])
                nc.vector.reduce_sum(R[:, o:o + 1], p0[:],
                                     axis=mybir.AxisListType.X)
            c_ps = ps_g.tile([1, OC], F32, tag="g")
            nc.tensor.matmul(c_ps[:], lhsT=ones_c2[:], rhs=R[:],
                             start=True, stop=True)
            c_sb = apool.tile([1, OC], F32, tag="c_sb")
            nc.vector.tensor_add(c_sb[:], c_ps[:], fbt[:])
            nc.sync.dma_start(hC[:], c_sb[:])

    _split_multi_waits(nc, tmpl)
    return nc


def _prep_inputs(feats, c, q_w1, q_b1, q_w2, q_b2, fcc_w, fcc_b):
    """Shard + lay out host-side. Returns list of per-core input dicts."""
    bf16 = ml_dtypes.bfloat16
    feats = np.asarray(feats, np.float32)
    c = np.asarray(c, np.float32)
    w1 = np.ascontiguousarray(np.asarray(q_w1, np.float32))
    w2 = np.ascontiguousarray(np.asarray(q_w2, np.float32))
    b1 = np.asarray(q_b1, np.float32).reshape(HID, 1)
    b2 = np.asarray(q_b2, np.float32).reshape(HID, 1)
    fw = np.ascontiguousarray(np.asarray(fcc_w, np.float32))
    fb = np.asarray(fcc_b, np.float32).reshape(1, OC)
    w1b = w1.astype(bf16)
    w2b = w2.astype(bf16)

    in_maps = []
    for core in range(N_CORES):
        b, h = divmod(core, 2)
        S = feats[b, h * N_LOC:(h + 1) * N_LOC]          # [20000, 512] view
        F = np.zeros((N_PAD, DIM), bf16)
        F[:N_LOC] = S
        FT = np.zeros((DIM, N_PAD), bf16)
        FT[:, :N_LOC] = S.T
        top = np.argmax(c[b], axis=0)                     # [2] shard-time index
        mT = np.ascontiguousarray(feats[b, top].T)        # [512, 2] f32
        in_maps.append({
            "F": F, "FT": FT, "mT": mT,
            "w1b": w1b, "w2b": w2b, "w1f": w1, "w2f": w2,
            "b1": b1, "b2": b2, "fw": fw, "fb": fb,
        })
    return in_maps


def run(inputs, trace=False):
    if "nc" not in _CACHE:
        _CACHE["nc"] = _build_nc()
    nc = _CACHE["nc"]
    in_maps = _prep_inputs(**inputs)
    res = run_bass_kernel_spmd(nc, in_maps, core_ids=list(range(N_CORES)),
                               trace=trace)
    A = np.empty((BATCH, INST, OC), np.float32)
    B = np.empty((BATCH, OC, DIM), np.float32)
    C = np.empty((BATCH, OC), np.float32)
    for b in range(BATCH):
        r0 = res.results[2 * b]
        r1 = res.results[2 * b + 1]
        A[b, :N_LOC] = r0["A_out"][:N_LOC]
        A[b, N_LOC:] = r1["A_out"][:N_LOC]
        B[b] = r0["B_out"]
        C[b] = r0["C_out"][0]
    return (C, A, B), res


def kernel(**inputs):
    out, _ = run(inputs, trace=False)
    return out


# revision 3
# speedup vs baseline: 1.4844x; 1.4844x over previous
"""Trainium2 Bass kernel for nn_BClassifier (MIL attention pooling).

Reference computation (per bag b of BATCH=4, INST=40000 instances, DIM=512):
    Q = tanh(relu(feats @ w1 + b1) @ w2 + b2)                 # [n, 128]
    top = argmax(c[:, o]) per class o                          # instance index
    q_max = q_mlp(feats[top])                                  # [2, 128]
    A = softmax(Q @ q_max.T / sqrt(n), axis=instances)         # [n, 2]
    B = A.T @ feats                                            # [2, 512]
    C = einsum('id,oid->o', B, fcc_w) + fcc_b                  # [2]
    returns (C, A, B)

Sharding: bags x halves -> 8 cores. Core c handles bag b=c//2, instance half
h=c%2 (20000 instances). The softmax normalizer Z and the unnormalized
B = sum_n exp(s_n) * feats_n are partial-summed per core and combined with a
2-rank AllGather per bag pair; each core then normalizes its half of A on
device. The argmax of c (tiny index computation) is resolved at shard time on
the host, which replicates the winning instances' features to both cores of
the bag (the device runs the q_mlp on them).

Compute runs in bf16 on the TensorEngine (accumulation in fp32 PSUM). Since
the PE contracts over the partition axis, the MLP needs feats with the feature
dim on partitions while the B-accumulation needs the instance dim on
partitions; the host therefore ships the per-core feats slice in both layouts
(bf16), which costs the same HBM bytes as a single f32 layout: the kernel
stays at the ~41MB/core memory roofline. Both layouts are partition-blocked
([tile][128 part][4 blk][512]) so every DMA moves 4KB-contiguous runs per
partition.

exp(s) needs no max subtraction: |s| <= 128/sqrt(40000) = 0.64 by construction
(tanh-bounded dot products), so softmax is computed as exp(s)/sum(exp(s)).

The A output is written in the device-native [128][tile*4][2] layout (one
contiguous DMA); the host unscrambles it while gathering shards.
"""

import copy

import numpy as np
import ml_dtypes

import concourse.bass as bass
import concourse.mybir as mybir
import concourse.tile as tile
from concourse.bass_utils import run_bass_kernel_spmd

BATCH = 4
INST = 40000
DIM = 512
HID = 128
OC = 2
N_CORES = 8

N_LOC = INST // 2            # 20000 instances per core
NT = 512                     # rows per tile
N_TILES = 40                 # ceil(20000 / 512)
N_PAD = N_TILES * NT         # 20480
NBLK = N_TILES * 4           # 160 blocks of 128 rows
LAST_ROWS = N_LOC - (N_TILES - 1) * NT    # 32 valid rows in last tile
SCALE = 1.0 / np.sqrt(np.float32(INST))   # 1/200

F32 = mybir.dt.float32
BF16 = mybir.dt.bfloat16

_CACHE = {}


def _split_multi_waits(nc, tmpl):
    """Walrus in this container only accepts one sync-wait per instruction.
    Split any multi-wait instruction into single-wait EventSemaphore preludes
    on the same engine (engine sequencers execute in order, so gating is
    preserved)."""
    n = 0
    for bb in nc.main_func.blocks:
        out = []
        for inst in bb.instructions:
            si = inst.sync_info
            if si is not None and si.on_wait is not None and len(si.on_wait) > 1:
                waits = list(si.on_wait)
                for w in waits[:-1]:
                    nop = copy.deepcopy(tmpl)
                    n += 1
                    nop.name = f"I-wsplit-{n}"
                    nop.engine = inst.engine
                    nop.sync_info = mybir.SyncInfo(on_wait=[w], on_update=[])
                    out.append(nop)
                inst.sync_info = mybir.SyncInfo(
                    on_wait=[waits[-1]], on_update=list(si.on_update)
                )
            out.append(inst)
        bb.instructions = out
    return n


def _build_nc():
    nc = bass.Bass("TRN2", target_bir_lowering=False, debug=False,
                   num_devices=N_CORES)

    # template wait instruction for _split_multi_waits
    with nc.semaphore() as _s:
        _ti = nc.vector.wait_ge(_s, 1)
        tmpl = copy.deepcopy(_ti.ins)
    for bb in nc.main_func.blocks:
        bb.instructions = [i for i in bb.instructions if i.name != _ti.ins.name]

    # ---- I/O ----
    # F[t][p][i][d]  = feats_row(t*512 + i*128 + p)[d]        (natural rows)
    # FT[t][p][i][n] = feats_row(t*512 + n)[i*128 + p]        (transposed)
    hF = nc.dram_tensor("F", [N_TILES, 128, 4, DIM], BF16, kind="ExternalInput")
    hFT = nc.dram_tensor("FT", [N_TILES, 128, 4, NT], BF16, kind="ExternalInput")
    hmT = nc.dram_tensor("mT", [DIM, OC], F32, kind="ExternalInput")
    hw1b = nc.dram_tensor("w1b", [DIM, HID], BF16, kind="ExternalInput")
    hw2b = nc.dram_tensor("w2b", [HID, HID], BF16, kind="ExternalInput")
    hw1f = nc.dram_tensor("w1f", [DIM, HID], F32, kind="ExternalInput")
    hw2f = nc.dram_tensor("w2f", [HID, HID], F32, kind="ExternalInput")
    hb1 = nc.dram_tensor("b1", [HID, 1], F32, kind="ExternalInput")
    hb2 = nc.dram_tensor("b2", [HID, 1], F32, kind="ExternalInput")
    hfw = nc.dram_tensor("fw", [OC, OC, DIM], F32, kind="ExternalInput")
    hfb = nc.dram_tensor("fb", [1, OC], F32, kind="ExternalInput")

    # A in device-native layout [part][block][class]; host unscrambles
    hA = nc.dram_tensor("A_out", [128, NBLK, OC], F32, kind="ExternalOutput")
    hB = nc.dram_tensor("B_out", [OC, DIM], F32, kind="ExternalOutput")
    hC = nc.dram_tensor("C_out", [1, OC], F32, kind="ExternalOutput")

    with tile.TileContext(nc) as tc:
        with (
            tc.tile_pool(name="const", bufs=1) as cpool,
            tc.tile_pool(name="stage", bufs=1) as spool,
            tc.tile_pool(name="fb_pool", bufs=4) as fpool,
            tc.tile_pool(name="ft_pool", bufs=4) as ftpool,
            tc.tile_pool(name="act", bufs=3) as apool,
            tc.tile_pool(name="ps_h", bufs=2, space="PSUM") as ps_h,
            tc.tile_pool(name="ps_g", bufs=2, space="PSUM") as ps_g,
            tc.tile_pool(name="ps_s", bufs=2, space="PSUM") as ps_s,
            tc.tile_pool(name="ps_acc", bufs=1, space="PSUM") as ps_acc,
            tc.tile_pool(name="dram", bufs=1, space="DRAM") as dram,
        ):
            # ---- constants into SBUF ----
            w1b = cpool.tile([128, 4, HID], BF16, tag="w1b")
            nc.sync.dma_start(w1b[:], hw1b.ap().rearrange("(k p) h -> p k h", p=128))
            w2b = cpool.tile([128, HID], BF16, tag="w2b")
            nc.sync.dma_start(w2b[:], hw2b[:])
            w1f = cpool.tile([128, 4, HID], F32, tag="w1f")
            nc.sync.dma_start(w1f[:], hw1f.ap().rearrange("(k p) h -> p k h", p=128))
            w2f = cpool.tile([128, HID], F32, tag="w2f")
            nc.sync.dma_start(w2f[:], hw2f[:])
            mT = cpool.tile([128, 4, OC], F32, tag="mT")
            nc.sync.dma_start(mT[:], hmT.ap().rearrange("(k p) o -> p k o", p=128))
            b1 = cpool.tile([HID, 1], F32, tag="b1")
            nc.sync.dma_start(b1[:], hb1[:])
            b2 = cpool.tile([HID, 1], F32, tag="b2")
            nc.sync.dma_start(b2[:], hb2[:])
            fw = cpool.tile([OC, OC, DIM], F32, tag="fw")
            nc.sync.dma_start(fw[:], hfw.ap().rearrange("o i d -> i o d"))
            fbt = cpool.tile([1, OC], F32, tag="fbt")
            nc.sync.dma_start(fbt[:], hfb[:])

            ones_bf = cpool.tile([128, 1], BF16, tag="ones_bf")
            nc.vector.memset(ones_bf[:], 1.0)
            ones_r = cpool.tile([1, 128], F32, tag="ones_r")
            nc.vector.memset(ones_r[:], 1.0)
            ones_c2 = cpool.tile([2, 1], F32, tag="ones_c2")
            nc.vector.memset(ones_c2[:], 1.0)

            # A staging buffer: e = exp(s) for all local blocks, f32
            Asb = spool.tile([128, NBLK, OC], F32, tag="Asb")

            # ---- q_max chain (f32, one-time) ----
            hm_ps = ps_h.tile([128, OC], F32, tag="h")
            for k in range(4):
                nc.tensor.matmul(hm_ps[:], lhsT=w1f[:, k, :], rhs=mT[:, k, :],
                                 start=(k == 0), stop=(k == 3))
            hm = apool.tile([128, OC], F32, tag="hm")
            nc.scalar.activation(hm[:], hm_ps[:],
                                 mybir.ActivationFunctionType.Relu, bias=b1[:])
            gm_ps = ps_g.tile([128, OC], F32, tag="g")
            nc.tensor.matmul(gm_ps[:], lhsT=w2f[:], rhs=hm[:], start=True, stop=True)
            qmT_f = apool.tile([128, OC], F32, tag="qmT_f")
            nc.scalar.activation(qmT_f[:], gm_ps[:],
                                 mybir.ActivationFunctionType.Tanh, bias=b2[:])
            qmT = cpool.tile([128, OC], BF16, tag="qmT")
            nc.vector.tensor_copy(qmT[:], qmT_f[:])

            # ---- accumulators (persist across the whole loop) ----
            B_ps = ps_acc.tile([OC, DIM], F32, tag="B")
            Z_ps = ps_acc.tile([OC, 1], F32, tag="Z")

            # ---- main loop over 512-row tiles ----
            for t in range(N_TILES):
                nblocks = 4 if t < N_TILES - 1 else 1
                kk = 128 if t < N_TILES - 1 else LAST_ROWS

                Fb = fpool.tile([128, 4, DIM], BF16, tag="Fb")
                nc.sync.dma_start(Fb[:], hF[t])
                FT = ftpool.tile([128, 4, NT], BF16, tag="FT")
                nc.sync.dma_start(FT[:], hFT[t])

                # h^T = relu(w1^T F^T + b1): [128h, 512n]
                h_ps = ps_h.tile([HID, NT], F32, tag="h")
                for k in range(4):
                    nc.tensor.matmul(h_ps[:], lhsT=w1b[:, k, :], rhs=FT[:, k, :],
                                     start=(k == 0), stop=(k == 3))
                h_bf = apool.tile([HID, NT], BF16, tag="h_bf")
                # (x + b1) then max(,0) on DVE (keeps ACT for tanh/exp only)
                nc.vector.tensor_scalar(h_bf[:], h_ps[:], b1[:], 0.0,
                                        op0=mybir.AluOpType.add,
                                        op1=mybir.AluOpType.max)

                # Q^T = tanh(w2^T h^T + b2): [128k, 512n]
                g_ps = ps_g.tile([HID, NT], F32, tag="g")
                nc.tensor.matmul(g_ps[:], lhsT=w2b[:], rhs=h_bf[:],
                                 start=True, stop=True)
                q_bf = apool.tile([HID, NT], BF16, tag="q_bf")
                nc.scalar.activation(q_bf[:], g_ps[:],
                                     mybir.ActivationFunctionType.Tanh, bias=b2[:])

                # s = Q @ q_max^T per 128-row block: [128, 4, 2]
                s_ps = ps_s.tile([128, 4, OC], F32, tag="s")
                for i in range(4):
                    nc.tensor.matmul(s_ps[:, i, :], lhsT=q_bf[:, i * 128:(i + 1) * 128],
                                     rhs=qmT[:], start=True, stop=True,
                                     skip_group_check=True)

                # e = exp(s/200): f32 into the A staging buffer (ACT),
                # bf16 copy for the PE accumulators (DVE cast)
                nc.scalar.activation(Asb[:, t * 4:t * 4 + 4, :], s_ps[:],
                                     mybir.ActivationFunctionType.Exp,
                                     scale=float(SCALE))
                e_bf = apool.tile([128, 4, OC], BF16, tag="e_bf")
                nc.vector.tensor_copy(e_bf[:], Asb[:, t * 4:t * 4 + 4, :])

                # B += e^T F ; Z += e^T 1 (contract instances, per 128-block)
                for i in range(nblocks):
                    last = (t == N_TILES - 1) and (i == nblocks - 1)
                    first = (t == 0) and (i == 0)
                    nc.tensor.matmul(B_ps[:], lhsT=e_bf[:kk, i, :],
                                     rhs=Fb[:kk, i, :],
                                     start=first, stop=last,
                                     skip_group_check=True)
                    nc.tensor.matmul(Z_ps[:], lhsT=e_bf[:kk, i, :],
                                     rhs=ones_bf[:kk, :],
                                     start=first, stop=last,
                                     skip_group_check=True)

            # ---- combine partials across the bag's core pair (AllGather) ----
            B_sb = apool.tile([OC, DIM], F32, tag="B_sb")
            nc.vector.tensor_copy(B_sb[:], B_ps[:])
            Z_sb = apool.tile([OC, 1], F32, tag="Z_sb")
            nc.vector.tensor_copy(Z_sb[:], Z_ps[:])

            cc_in = dram.tile([OC, DIM + 1], F32, tag="cc_in")
            cc_out = dram.tile([2 * OC, DIM + 1], F32, tag="cc_out")
            nc.sync.dma_start(cc_in[:, 0:1], Z_sb[:])
            nc.sync.dma_start(cc_in[:, 1:DIM + 1], B_sb[:])
            nc.gpsimd.collective_compute(
                "AllGather",
                mybir.AluOpType.bypass,
                replica_groups=[[0, 1], [2, 3], [4, 5], [6, 7]],
                ins=[cc_in.opt()],
                outs=[cc_out.opt()],
            )

            # sum the two ranks' partials: [2, 513] + [2, 513]
            ZB0 = apool.tile([OC, DIM + 1], F32, tag="ZB0")
            nc.sync.dma_start(ZB0[:], cc_out[0:OC, :])
            ZB1 = apool.tile([OC, DIM + 1], F32, tag="ZB1")
            nc.sync.dma_start(ZB1[:], cc_out[OC:2 * OC, :])
            ZBg = apool.tile([OC, DIM + 1], F32, tag="ZBg")
            nc.vector.tensor_add(ZBg[:], ZB0[:], ZB1[:])
            # transposed copy of Z for the partition broadcast
            Zg_r = apool.tile([1, OC], F32, tag="Zg_r")
            nc.sync.dma_start(Zg_r[:], cc_out[0:OC, 0:1].rearrange("o x -> x o"))
            Z1_r = apool.tile([1, OC], F32, tag="Z1_r")
            nc.sync.dma_start(Z1_r[:], cc_out[OC:2 * OC, 0:1].rearrange("o x -> x o"))

            zi_c = apool.tile([OC, 1], F32, tag="zi_c")
            nc.vector.reciprocal(zi_c[:], ZBg[:, 0:1])
            Zs_r = apool.tile([1, OC], F32, tag="Zs_r")
            nc.vector.tensor_add(Zs_r[:], Zg_r[:], Z1_r[:])
            zi_r = apool.tile([1, OC], F32, tag="zi_r")
            nc.vector.reciprocal(zi_r[:], Zs_r[:])

            # B_out = B_glob / Z  -> HBM
            Bn = apool.tile([OC, DIM], F32, tag="Bn")
            nc.vector.tensor_scalar_mul(Bn[:], ZBg[:, 1:DIM + 1], zi_c[:])
            nc.sync.dma_start(hB[:], Bn[:])

            # A = e / Z : broadcast 1/Z down the partitions via rank-1 matmul
            zb_ps = ps_s.tile([128, OC], F32, tag="s")
            nc.tensor.matmul(zb_ps[:], lhsT=ones_r[:], rhs=zi_r[:],
                             start=True, stop=True)
            zb = apool.tile([128, OC], F32, tag="zb")
            nc.vector.tensor_copy(zb[:], zb_ps[:])
            nc.vector.tensor_mul(Asb[:], Asb[:],
                                 zb[:, None, :].broadcast_to([128, NBLK, OC]))
            nc.sync.dma_start(hA[:], Asb[:])

            # C = einsum('id,oid->o', B_out, fcc_w) + fcc_b
            R = apool.tile([OC, OC], F32, tag="R")
            p0 = apool.tile([OC, DIM], F32, tag="p0")
            for o in range(OC):
                nc.vector.tensor_mul(p0[:], Bn[:], fw[:, o, :])
                nc.vector.reduce_sum(R[:, o:o + 1], p0[:],
                                     axis=mybir.AxisListType.X)
            c_ps = ps_g.tile([1, OC], F32, tag="g")
            nc.tensor.matmul(c_ps[:], lhsT=ones_c2[:], rhs=R[:],
                             start=True, stop=True)
            c_sb = apool.tile([1, OC], F32, tag="c_sb")
            nc.vector.tensor_add(c_sb[:], c_ps[:], fbt[:])
            nc.sync.dma_start(hC[:], c_sb[:])

    _split_multi_waits(nc, tmpl)
    return nc


def _prep_inputs(feats, c, q_w1, q_b1, q_w2, q_b2, fcc_w, fcc_b):
    """Shard + lay out host-side. Returns list of per-core input dicts."""
    bf16 = ml_dtypes.bfloat16
    feats = np.asarray(feats, np.float32)
    c = np.asarray(c, np.float32)
    w1 = np.ascontiguousarray(np.asarray(q_w1, np.float32))
    w2 = np.ascontiguousarray(np.asarray(q_w2, np.float32))
    b1 = np.asarray(q_b1, np.float32).reshape(HID, 1)
    b2 = np.asarray(q_b2, np.float32).reshape(HID, 1)
    fw = np.ascontiguousarray(np.asarray(fcc_w, np.float32))
    fb = np.asarray(fcc_b, np.float32).reshape(1, OC)
    w1b = w1.astype(bf16)
    w2b = w2.astype(bf16)

    in_maps = []
    for core in range(N_CORES):
        b, h = divmod(core, 2)
        S = feats[b, h * N_LOC:(h + 1) * N_LOC]          # [20000, 512] view
        Sp = np.zeros((N_PAD, DIM), bf16)
        Sp[:N_LOC] = S
        # F[t][p][i][d] = Sp[t*512 + i*128 + p][d]
        F = np.ascontiguousarray(
            Sp.reshape(N_TILES, 4, 128, DIM).transpose(0, 2, 1, 3))
        # FT[t][p][i][n] = Sp[t*512 + n][i*128 + p]
        FT = np.ascontiguousarray(
            Sp.reshape(N_TILES, NT, 4, 128).transpose(0, 3, 2, 1))
        top = np.argmax(c[b], axis=0)                     # [2] shard-time index
        mT = np.ascontiguousarray(feats[b, top].T)        # [512, 2] f32
        in_maps.append({
            "F": F, "FT": FT, "mT": mT,
            "w1b": w1b, "w2b": w2b, "w1f": w1, "w2f": w2,
            "b1": b1, "b2": b2, "fw": fw, "fb": fb,
        })
    return in_maps


def run(inputs, trace=False):
    if "nc" not in _CACHE:
        _CACHE["nc"] = _build_nc()
    nc = _CACHE["nc"]
    in_maps = _prep_inputs(**inputs)
    res = run_bass_kernel_spmd(nc, in_maps, core_ids=list(range(N_CORES)),
                               trace=trace)
    A = np.empty((BATCH, INST, OC), np.float32)
    B = np.empty((BATCH, OC, DIM), np.float32)
    C = np.empty((BATCH, OC), np.float32)
    for b in range(BATCH):
        r0 = res.results[2 * b]
        r1 = res.results[2 * b + 1]
        # A_out is [128 part][block][class]; row n = block*128 + part
        A[b, :N_LOC] = r0["A_out"].transpose(1, 0, 2).reshape(N_PAD, OC)[:N_LOC]
        A[b, N_LOC:] = r1["A_out"].transpose(1, 0, 2).reshape(N_PAD, OC)[:N_LOC]
        B[b] = r0["B_out"]
        C[b] = r0["C_out"][0]
    return (C, A, B), res


def kernel(**inputs):
    out, _ = run(inputs, trace=False)
    return out


# revision 6
# speedup vs baseline: 2.3327x; 1.5715x over previous
"""Trainium2 Bass kernel for nn_BClassifier (MIL attention pooling).

Reference computation (per bag b of BATCH=4, INST=40000 instances, DIM=512):
    Q = tanh(relu(feats @ w1 + b1) @ w2 + b2)                 # [n, 128]
    top = argmax(c[:, o]) per class o                          # instance index
    q_max = q_mlp(feats[top])                                  # [2, 128]
    A = softmax(Q @ q_max.T / sqrt(n), axis=instances)         # [n, 2]
    B = A.T @ feats                                            # [2, 512]
    C = einsum('id,oid->o', B, fcc_w) + fcc_b                  # [2]
    returns (C, A, B)

Sharding: bags x halves -> 8 cores. Core c handles bag b=c//2, instance half
h=c%2 (20000 instances). The softmax normalizer Z and the unnormalized
B = sum_n exp(s_n) * feats_n are partial-summed per core and combined with a
2-rank AllGather per bag pair; each core then normalizes its half of A on
device. The argmax of c (tiny index computation) is resolved at shard time on
the host, which replicates the winning instances' features to both cores of
the bag (the device runs the q_mlp on them).

Compute runs in bf16 on the TensorEngine (accumulation in fp32 PSUM). Since
the PE contracts over the partition axis, the MLP needs feats with the feature
dim on partitions while the B-accumulation needs the instance dim on
partitions; the host therefore ships the per-core feats slice in both layouts
(bf16), which costs the same HBM bytes as a single f32 layout: the kernel
stays at the ~41MB/core memory roofline. Both layouts are partition-blocked
([tile][128 part][4 blk][512]) so every DMA moves 4KB-contiguous runs per
partition.

exp(s) needs no max subtraction: |s| <= 128/sqrt(40000) = 0.64 by construction
(tanh-bounded dot products), so softmax is computed as exp(s)/sum(exp(s)).

The A output is written in the device-native [128][tile*4][2] layout (one
contiguous DMA); the host unscrambles it while gathering shards.
"""

import copy

import numpy as np
import ml_dtypes

import concourse.bass as bass
import concourse.mybir as mybir
import concourse.tile as tile
from concourse.bass_utils import run_bass_kernel_spmd

BATCH = 4
INST = 40000
DIM = 512
HID = 128
OC = 2
N_CORES = 8

N_LOC = INST // 2            # 20000 instances per core
NT = 512                     # rows per tile
N_TILES = 40                 # ceil(20000 / 512)
N_PAD = N_TILES * NT         # 20480
NBLK = N_TILES * 4           # 160 blocks of 128 rows
LAST_ROWS = N_LOC - (N_TILES - 1) * NT    # 32 valid rows in last tile
SCALE = 1.0 / np.sqrt(np.float32(INST))   # 1/200

F32 = mybir.dt.float32
BF16 = mybir.dt.bfloat16

_CACHE = {}


def _split_multi_waits(nc, tmpl):
    """Walrus in this container only accepts one sync-wait per instruction.
    Split any multi-wait instruction into single-wait EventSemaphore preludes
    on the same engine (engine sequencers execute in order, so gating is
    preserved)."""
    n = 0
    for bb in nc.main_func.blocks:
        out = []
        for inst in bb.instructions:
            si = inst.sync_info
            if si is not None and si.on_wait is not None and len(si.on_wait) > 1:
                waits = list(si.on_wait)
                for w in waits[:-1]:
                    nop = copy.deepcopy(tmpl)
                    n += 1
                    nop.name = f"I-wsplit-{n}"
                    nop.engine = inst.engine
                    nop.sync_info = mybir.SyncInfo(on_wait=[w], on_update=[])
                    out.append(nop)
                inst.sync_info = mybir.SyncInfo(
                    on_wait=[waits[-1]], on_update=list(si.on_update)
                )
            out.append(inst)
        bb.instructions = out
    return n


def _build_nc():
    nc = bass.Bass("TRN2", target_bir_lowering=False, debug=False,
                   num_devices=N_CORES)

    # template wait instruction for _split_multi_waits
    with nc.semaphore() as _s:
        _ti = nc.vector.wait_ge(_s, 1)
        tmpl = copy.deepcopy(_ti.ins)
    for bb in nc.main_func.blocks:
        bb.instructions = [i for i in bb.instructions if i.name != _ti.ins.name]

    # ---- I/O ----
    # F[t][p][i][d]  = feats_row(t*512 + i*128 + p)[d]        (natural rows)
    # FT[t][p][i][n] = feats_row(t*512 + n)[i*128 + p]        (transposed)
    hF = nc.dram_tensor("F", [N_TILES, 128, 4, DIM], BF16, kind="ExternalInput")
    hFT = nc.dram_tensor("FT", [N_TILES, 128, 4, NT], BF16, kind="ExternalInput")
    hmT = nc.dram_tensor("mT", [DIM, OC], F32, kind="ExternalInput")
    hw1b = nc.dram_tensor("w1b", [DIM, HID], BF16, kind="ExternalInput")
    hw2b = nc.dram_tensor("w2b", [HID, HID], BF16, kind="ExternalInput")
    hw1f = nc.dram_tensor("w1f", [DIM, HID], F32, kind="ExternalInput")
    hw2f = nc.dram_tensor("w2f", [HID, HID], F32, kind="ExternalInput")
    hb1 = nc.dram_tensor("b1", [HID, 1], F32, kind="ExternalInput")
    hb2 = nc.dram_tensor("b2", [HID, 1], F32, kind="ExternalInput")
    hfw = nc.dram_tensor("fw", [OC, OC, DIM], F32, kind="ExternalInput")
    hfb = nc.dram_tensor("fb", [1, OC], F32, kind="ExternalInput")

    # A in device-native layout [part][block][class]; host unscrambles
    hA = nc.dram_tensor("A_out", [128, NBLK, OC], F32, kind="ExternalOutput")
    hB = nc.dram_tensor("B_out", [OC, DIM], F32, kind="ExternalOutput")
    hC = nc.dram_tensor("C_out", [1, OC], F32, kind="ExternalOutput")

    with tile.TileContext(nc) as tc:
        with (
            tc.tile_pool(name="const", bufs=1) as cpool,
            tc.tile_pool(name="stage", bufs=1) as spool,
            tc.tile_pool(name="fb_pool", bufs=6) as fpool,
            tc.tile_pool(name="ft_pool", bufs=4) as ftpool,
            tc.tile_pool(name="act", bufs=3) as apool,
            tc.tile_pool(name="ps_h", bufs=2, space="PSUM") as ps_h,
            tc.tile_pool(name="ps_g", bufs=2, space="PSUM") as ps_g,
            tc.tile_pool(name="ps_s", bufs=2, space="PSUM") as ps_s,
            tc.tile_pool(name="ps_acc", bufs=1, space="PSUM") as ps_acc,
            tc.tile_pool(name="dram", bufs=1, space="DRAM") as dram,
        ):
            # ---- constants into SBUF ----
            w1b = cpool.tile([128, 4, HID], BF16, tag="w1b")
            nc.sync.dma_start(w1b[:], hw1b.ap().rearrange("(k p) h -> p k h", p=128))
            w2b = cpool.tile([128, HID], BF16, tag="w2b")
            nc.sync.dma_start(w2b[:], hw2b[:])
            w1f = cpool.tile([128, 4, HID], F32, tag="w1f")
            nc.sync.dma_start(w1f[:], hw1f.ap().rearrange("(k p) h -> p k h", p=128))
            w2f = cpool.tile([128, HID], F32, tag="w2f")
            nc.sync.dma_start(w2f[:], hw2f[:])
            mT = cpool.tile([128, 4, OC], F32, tag="mT")
            nc.sync.dma_start(mT[:], hmT.ap().rearrange("(k p) o -> p k o", p=128))
            b1 = cpool.tile([HID, 1], F32, tag="b1")
            nc.sync.dma_start(b1[:], hb1[:])
            b2 = cpool.tile([HID, 1], F32, tag="b2")
            nc.sync.dma_start(b2[:], hb2[:])
            fw = cpool.tile([OC, OC, DIM], F32, tag="fw")
            nc.sync.dma_start(fw[:], hfw.ap().rearrange("o i d -> i o d"))
            fbt = cpool.tile([1, OC], F32, tag="fbt")
            nc.sync.dma_start(fbt[:], hfb[:])

            ones_bf = cpool.tile([128, 1], BF16, tag="ones_bf")
            nc.vector.memset(ones_bf[:], 1.0)
            ones_r = cpool.tile([1, 128], F32, tag="ones_r")
            nc.vector.memset(ones_r[:], 1.0)
            ones_c2 = cpool.tile([2, 1], F32, tag="ones_c2")
            nc.vector.memset(ones_c2[:], 1.0)

            # A staging buffer: e = exp(s) for all local blocks, f32
            Asb = spool.tile([128, NBLK, OC], F32, tag="Asb")

            # ---- q_max chain (f32, one-time) ----
            hm_ps = ps_h.tile([128, OC], F32, tag="h")
            for k in range(4):
                nc.tensor.matmul(hm_ps[:], lhsT=w1f[:, k, :], rhs=mT[:, k, :],
                                 start=(k == 0), stop=(k == 3))
            hm = apool.tile([128, OC], F32, tag="hm")
            nc.scalar.activation(hm[:], hm_ps[:],
                                 mybir.ActivationFunctionType.Relu, bias=b1[:])
            gm_ps = ps_g.tile([128, OC], F32, tag="g")
            nc.tensor.matmul(gm_ps[:], lhsT=w2f[:], rhs=hm[:], start=True, stop=True)
            qmT_f = apool.tile([128, OC], F32, tag="qmT_f")
            nc.scalar.activation(qmT_f[:], gm_ps[:],
                                 mybir.ActivationFunctionType.Tanh, bias=b2[:])
            qmT = cpool.tile([128, OC], BF16, tag="qmT")
            nc.vector.tensor_copy(qmT[:], qmT_f[:])

            # ---- accumulators (persist across the whole loop) ----
            B_ps = ps_acc.tile([OC, DIM], F32, tag="B")
            Z_ps = ps_acc.tile([OC, 1], F32, tag="Z")

            # ---- main loop over 512-row tiles, software-pipelined ----
            # stage skew: L1(u) | relu/L2/tanh(u-1) | s/exp/cast(u-2) | B,Z(u-3)
            # so every PE instruction's inputs were produced >= 1 iteration
            # earlier and the PE never stalls on the ACT/DVE round-trips.
            Fb_t, FT_t, hps_t, hbf_t, gps_t, qbf_t, sps_t, ebf_t = \
                {}, {}, {}, {}, {}, {}, {}, {}
            for u in range(N_TILES + 3):
                if u < N_TILES:
                    Fb = fpool.tile([128, 4, DIM], BF16, tag="Fb")
                    nc.sync.dma_start(Fb[:], hF[u])
                    Fb_t[u] = Fb
                    FT = ftpool.tile([128, 4, NT], BF16, tag="FT")
                    nc.sync.dma_start(FT[:], hFT[u])
                    FT_t[u] = FT
                    # h^T = w1^T F^T: [128h, 512n]
                    h_ps = ps_h.tile([HID, NT], F32, tag="h")
                    for k in range(4):
                        nc.tensor.matmul(h_ps[:], lhsT=w1b[:, k, :],
                                         rhs=FT[:, k, :],
                                         start=(k == 0), stop=(k == 3))
                    hps_t[u] = h_ps

                if 1 <= u <= N_TILES:
                    t = u - 1
                    # relu(x + b1) on DVE (keeps ACT for tanh/exp only)
                    h_bf = apool.tile([HID, NT], BF16, tag="h_bf")
                    nc.vector.tensor_scalar(h_bf[:], hps_t.pop(t)[:], b1[:], 0.0,
                                            op0=mybir.AluOpType.add,
                                            op1=mybir.AluOpType.max)
                    hbf_t[t] = h_bf
                    # Q^T = tanh(w2^T h^T + b2): [128k, 512n]
                    g_ps = ps_g.tile([HID, NT], F32, tag="g")
                    nc.tensor.matmul(g_ps[:], lhsT=w2b[:], rhs=hbf_t.pop(t)[:],
                                     start=True, stop=True)
                    q_bf = apool.tile([HID, NT], BF16, tag="q_bf")
                    nc.scalar.activation(q_bf[:], g_ps[:],
                                         mybir.ActivationFunctionType.Tanh,
                                         bias=b2[:])
                    qbf_t[t] = q_bf

                if 2 <= u <= N_TILES + 1:
                    t = u - 2
                    # s = Q @ q_max^T per 128-row block: [128, 4, 2]
                    q_bf = qbf_t.pop(t)
                    s_ps = ps_s.tile([128, 4, OC], F32, tag="s")
                    for i in range(4):
                        nc.tensor.matmul(s_ps[:, i, :],
                                         lhsT=q_bf[:, i * 128:(i + 1) * 128],
                                         rhs=qmT[:], start=True, stop=True,
                                         skip_group_check=True)
                    # e = exp(s/200): f32 into the A staging buffer (ACT),
                    # bf16 copy for the PE accumulators (DVE cast)
                    nc.scalar.activation(Asb[:, t * 4:t * 4 + 4, :], s_ps[:],
                                         mybir.ActivationFunctionType.Exp,
                                         scale=float(SCALE))
                    e_bf = apool.tile([128, 4, OC], BF16, tag="e_bf")
                    nc.vector.tensor_copy(e_bf[:], Asb[:, t * 4:t * 4 + 4, :])
                    ebf_t[t] = e_bf

                if u >= 3:
                    t = u - 3
                    # B += e^T F ; Z += e^T 1 (contract instances, per block)
                    nblocks = 4 if t < N_TILES - 1 else 1
                    kk = 128 if t < N_TILES - 1 else LAST_ROWS
                    e_bf = ebf_t.pop(t)
                    Fb = Fb_t.pop(t)
                    for i in range(nblocks):
                        last = (t == N_TILES - 1) and (i == nblocks - 1)
                        first = (t == 0) and (i == 0)
                        nc.tensor.matmul(B_ps[:], lhsT=e_bf[:kk, i, :],
                                         rhs=Fb[:kk, i, :],
                                         start=first, stop=last,
                                         skip_group_check=True)
                        nc.tensor.matmul(Z_ps[:], lhsT=e_bf[:kk, i, :],
                                         rhs=ones_bf[:kk, :],
                                         start=first, stop=last,
                                         skip_group_check=True)

            # ---- combine partials across the bag's core pair (AllReduce) ----
            B_sb = apool.tile([OC, DIM], F32, tag="B_sb")
            nc.vector.tensor_copy(B_sb[:], B_ps[:])
            Z_sb = apool.tile([OC, 1], F32, tag="Z_sb")
            nc.vector.tensor_copy(Z_sb[:], Z_ps[:])

            cc_in = dram.tile([OC, DIM + 1], F32, tag="cc_in")
            cc_out = dram.tile([OC, DIM + 1], F32, tag="cc_out")
            nc.sync.dma_start(cc_in[:, 0:1], Z_sb[:])
            nc.sync.dma_start(cc_in[:, 1:DIM + 1], B_sb[:])
            nc.gpsimd.collective_compute(
                "AllReduce",
                mybir.AluOpType.add,
                replica_groups=[[0, 1], [2, 3], [4, 5], [6, 7]],
                ins=[cc_in.opt()],
                outs=[cc_out.opt()],
            )

            ZBg = apool.tile([OC, DIM + 1], F32, tag="ZBg")
            nc.sync.dma_start(ZBg[:], cc_out[:])
            # transposed copy of Z for the partition broadcast
            Zs_r = apool.tile([1, OC], F32, tag="Zs_r")
            nc.sync.dma_start(Zs_r[:], cc_out[:, 0:1].rearrange("o x -> x o"))

            zi_c = apool.tile([OC, 1], F32, tag="zi_c")
            nc.vector.reciprocal(zi_c[:], ZBg[:, 0:1])
            zi_r = apool.tile([1, OC], F32, tag="zi_r")
            nc.vector.reciprocal(zi_r[:], Zs_r[:])

            # B_out = B_glob / Z  -> HBM
            Bn = apool.tile([OC, DIM], F32, tag="Bn")
            nc.vector.tensor_scalar_mul(Bn[:], ZBg[:, 1:DIM + 1], zi_c[:])
            nc.sync.dma_start(hB[:], Bn[:])

            # A = e / Z : broadcast 1/Z down the partitions via rank-1 matmul
            zb_ps = ps_s.tile([128, OC], F32, tag="s")
            nc.tensor.matmul(zb_ps[:], lhsT=ones_r[:], rhs=zi_r[:],
                             start=True, stop=True)
            zb = apool.tile([128, OC], F32, tag="zb")
            nc.vector.tensor_copy(zb[:], zb_ps[:])
            nc.vector.tensor_mul(Asb[:], Asb[:],
                                 zb[:, None, :].broadcast_to([128, NBLK, OC]))
            nc.sync.dma_start(hA[:], Asb[:])

            # C = einsum('id,oid->o', B_out, fcc_w) + fcc_b
            R = apool.tile([OC, OC], F32, tag="R")
            p0 = apool.tile([OC, DIM], F32, tag="p0")
            for o in range(OC):
                nc.vector.tensor_mul(p0[:], Bn[:], fw[:, o, :])
                nc.vector.reduce_sum(R[:, o:o + 1], p0[:],
                                     axis=mybir.AxisListType.X)
            c_ps = ps_g.tile([1, OC], F32, tag="g")
            nc.tensor.matmul(c_ps[:], lhsT=ones_c2[:], rhs=R[:],
                             start=True, stop=True)
            c_sb = apool.tile([1, OC], F32, tag="c_sb")
            nc.vector.tensor_add(c_sb[:], c_ps[:], fbt[:])
            nc.sync.dma_start(hC[:], c_sb[:])

    _split_multi_waits(nc, tmpl)
    return nc


def _prep_inputs(feats, c, q_w1, q_b1, q_w2, q_b2, fcc_w, fcc_b):
    """Shard + lay out host-side. Returns list of per-core input dicts."""
    bf16 = ml_dtypes.bfloat16
    feats = np.asarray(feats, np.float32)
    c = np.asarray(c, np.float32)
    w1 = np.ascontiguousarray(np.asarray(q_w1, np.float32))
    w2 = np.ascontiguousarray(np.asarray(q_w2, np.float32))
    b1 = np.asarray(q_b1, np.float32).reshape(HID, 1)
    b2 = np.asarray(q_b2, np.float32).reshape(HID, 1)
    fw = np.ascontiguousarray(np.asarray(fcc_w, np.float32))
    fb = np.asarray(fcc_b, np.float32).reshape(1, OC)
    w1b = w1.astype(bf16)
    w2b = w2.astype(bf16)

    in_maps = []
    for core in range(N_CORES):
        b, h = divmod(core, 2)
        S = feats[b, h * N_LOC:(h + 1) * N_LOC]          # [20000, 512] view
        Sp = np.zeros((N_PAD, DIM), bf16)
        Sp[:N_LOC] = S
        # F[t][p][i][d] = Sp[t*512 + i*128 + p][d]
        F = np.ascontiguousarray(
            Sp.reshape(N_TILES, 4, 128, DIM).transpose(0, 2, 1, 3))
        # FT[t][p][i][n] = Sp[t*512 + n][i*128 + p]
        FT = np.ascontiguousarray(
            Sp.reshape(N_TILES, NT, 4, 128).transpose(0, 3, 2, 1))
        top = np.argmax(c[b], axis=0)                     # [2] shard-time index
        mT = np.ascontiguousarray(feats[b, top].T)        # [512, 2] f32
        in_maps.append({
            "F": F, "FT": FT, "mT": mT,
            "w1b": w1b, "w2b": w2b, "w1f": w1, "w2f": w2,
            "b1": b1, "b2": b2, "fw": fw, "fb": fb,
        })
    return in_maps


def run(inputs, trace=False):
    if "nc" not in _CACHE:
        _CACHE["nc"] = _build_nc()
    nc = _CACHE["nc"]
    in_maps = _prep_inputs(**inputs)
    res = run_bass_kernel_spmd(nc, in_maps, core_ids=list(range(N_CORES)),
                               trace=trace)
    A = np.empty((BATCH, INST, OC), np.float32)
    B = np.empty((BATCH, OC, DIM), np.float32)
    C = np.empty((BATCH, OC), np.float32)
    for b in range(BATCH):
        r0 = res.results[2 * b]
        r1 = res.results[2 * b + 1]
        # A_out is [128 part][block][class]; row n = block*128 + part
        A[b, :N_LOC] = r0["A_out"].transpose(1, 0, 2).reshape(N_PAD, OC)[:N_LOC]
        A[b, N_LOC:] = r1["A_out"].transpose(1, 0, 2).reshape(N_PAD, OC)[:N_LOC]
        B[b] = r0["B_out"]
        C[b] = r0["C_out"][0]
    return (C, A, B), res


def kernel(**inputs):
    out, _ = run(inputs, trace=False)
    return out
